# revision 26
# baseline (speedup 1.0000x reference)
"""Trainium2 Bass kernel for the e3nn-style uvu tensor product
(irreps 128x0e+128x1e+128x2e  x  1x1e, 6 paths, per-path u-weights).

Strategy (data-parallel over the batch axis N, 8 NeuronCores):
  out[z,k,u] = sum_t sign_t * x1[z, i_t, u] * H[z, b_t*128+u]
  H[z, :]    = x2row[z, :] @ R            (PE matmul, K=3, per 128-z tile)
  R[j, b*128+u] = |c_b| * w[p_b*128+u]    (host-built from w; b = (j,p,|c|) class)

Layout: z on SBUF partitions, (irrep k, channel u) along the free dim.
The 50 Wigner-3j terms become 50 tensor-tensor multiplies + 41 adds/subs,
split across the Vector and GpSimd engines; the Tensor engine builds H and
the Scalar engine moves H from PSUM to SBUF.
"""
import math
from math import factorial as _fact

import numpy as np

MUL = 128
PATHS = [(0, 1, 1), (1, 1, 0), (1, 1, 1), (1, 1, 2), (2, 1, 1), (2, 1, 2)]
SLBL = {0: 0, 1: 1, 2: 4}
N_CORES = 8
FREE = 9 * MUL  # 1152


# ---------------- Wigner 3j (identical math to the reference) ----------------
def _su2_cg(j1, j2, j3, m1, m2, m3):
    if m3 != m1 + m2:
        return 0.0
    vmin = max(-j1 + j2 + m3, -j1 + m1, 0)
    vmax = min(j2 + j3 + m1, j3 - j1 + j2, j3 + m3)
    f = _fact
    C = math.sqrt((2 * j3 + 1) * f(j3 + j1 - j2) * f(j3 - j1 + j2) * f(j1 + j2 - j3)
                  * f(j3 + m3) * f(j3 - m3)
                  / (f(j1 + j2 + j3 + 1) * f(j1 - m1) * f(j1 + m1) * f(j2 - m2) * f(j2 + m2)))
    S = 0.0
    for v in range(vmin, vmax + 1):
        S += (-1) ** (v + j2 + m2) * f(j2 + j3 + m1 - v) * f(j1 - m1 + v) \
             / (f(v) * f(j3 - j1 + j2 - v) * f(j3 + m3 - v) * f(v + j1 - j2 - m3))
    return C * S


def _change_basis_real_to_complex(l):
    q = np.zeros((2 * l + 1, 2 * l + 1), dtype=np.complex128)
    for m in range(-l, 0):
        q[l + m, l + abs(m)] = 1 / math.sqrt(2)
        q[l + m, l - abs(m)] = -1j / math.sqrt(2)
    q[l, l] = 1
    for m in range(1, l + 1):
        q[l + m, l + abs(m)] = (-1) ** m / math.sqrt(2)
        q[l + m, l - abs(m)] = 1j * (-1) ** m / math.sqrt(2)
    return (-1j) ** l * q


def _wigner_3j(l1, l2, l3):
    C = np.zeros((2 * l1 + 1, 2 * l2 + 1, 2 * l3 + 1))
    for m1 in range(-l1, l1 + 1):
        for m2 in range(-l2, l2 + 1):
            for m3 in range(-l3, l3 + 1):
                C[l1 + m1, l2 + m2, l3 + m3] = _su2_cg(l1, l2, l3, m1, m2, m3)
    Q1 = _change_basis_real_to_complex(l1)
    Q2 = _change_basis_real_to_complex(l2)
    Q3 = _change_basis_real_to_complex(l3)
    C = np.einsum('ij,kl,nm,ikn->jlm', Q1, Q2, Q3.conj(), C.astype(np.complex128))
    C = C.real
    return (C / np.linalg.norm(C)).astype(np.float32)


def _build_tables():
    terms = []
    for p, (l1, _l2, lo) in enumerate(PATHS):
        C = _wigner_3j(l1, 1, lo)
        for i in range(2 * l1 + 1):
            for j in range(3):
                for k in range(2 * lo + 1):
                    c = float(C[i, j, k])
                    if abs(c) > 1e-8:
                        terms.append((SLBL[lo] + k, SLBL[l1] + i, j, p, c))
    classes = sorted(set((j, p, round(abs(c), 7)) for (_, _, j, p, c) in terms))
    bid = {cl: i for i, cl in enumerate(classes)}
    perk = {k: [] for k in range(9)}
    for (kg, ig, j, p, c) in terms:
        perk[kg].append((ig, bid[(j, p, round(abs(c), 7))], 1 if c > 0 else -1))
    for k in range(9):
        perk[k].sort(key=lambda t: -t[2])
        assert perk[k][0][2] > 0
    return classes, perk


_CLASSES, _PERK = _build_tables()
NB = len(_CLASSES)  # 27


def _build_R(w):
    R = np.zeros((3, NB * MUL), dtype=np.float32)
    for b, (j, p, ac) in enumerate(_CLASSES):
        R[j, b * MUL:(b + 1) * MUL] = ac * w[p * MUL:(p + 1) * MUL]
    return R


# ---------------- Bass kernel ----------------
_CACHE = {}


def _plan_tiles(ns, zq):
    """[(z0, P, q)] tiles: q z-rows of P partitions each, z = z0 + qi*P + p."""
    tiles = []
    z0 = 0
    while ns - z0 >= zq * 128:
        tiles.append((z0, 128, zq))
        z0 += zq * 128
    while ns - z0 >= 128:
        tiles.append((z0, 128, 1))
        z0 += 128
    if ns - z0 > 0:
        tiles.append((z0, ns - z0, 1))
        z0 = ns
    return tiles


def _engine_split():
    """Hand-balanced per-k engine assignment (Vector ~2.5x GpSimd op rate)."""
    return {0: "v", 1: "v", 2: "v", 3: "v", 4: "g", 5: "v", 6: "v", 7: "v",
            8: "g"}


# k's whose accumulation runs as chain-adds on the elementwise engines
# (everything else accumulates on the Tensor engine via +/-I matmuls)
EW_ACCUM_K = {4, 5, 6, 8}
# contiguous runs of PE-accumulated k's for the PSUM->SBUF copies
PE_COPY_RUNS = [(0, 4), (7, 8)]


def _build_nc(ns, zq=2, cdt_name="float16"):
    """cdt_name: compute dtype for the elementwise term stage
    ("float16" ~5e-4 rel err, "bfloat16" ~4e-3, "float32" exact but slower).

    V3: the 50 term products go to a contiguous TMP slab (Vector/GpSimd);
    the per-k accumulation runs on the Tensor engine as +/-identity
    pass-through matmuls into PSUM (fp32), and the result is DMAed straight
    from PSUM to HBM.
    """
    import concourse.bacc as bacc
    import concourse.mybir as mybir
    from concourse.tile import TileContext

    f32 = mybir.dt.float32
    cdt = getattr(mybir.dt, cdt_name)
    mmdt = f32 if cdt == f32 else cdt  # matmul input dtype
    nc = bacc.Bacc("TRN2", target_bir_lowering=False, debug=False,
                   num_devices=N_CORES)
    x1 = nc.dram_tensor("x1", [ns, FREE], f32, kind="ExternalInput").ap()
    x2t = nc.dram_tensor("x2t", [3, ns], f32, kind="ExternalInput").ap()
    rmat = nc.dram_tensor("rmat", [3, NB * MUL], f32, kind="ExternalInput").ap()
    eye = nc.dram_tensor("eye", [128, 128], f32, kind="ExternalInput").ap()
    out = nc.dram_tensor("out", [ns, FREE], f32, kind="ExternalOutput").ap()

    tiles = _plan_tiles(ns, zq)
    assign = _engine_split()
    HW = NB * MUL  # H width per z-row: 3456
    W = zq * MUL   # free width of one (k) slab / one product

    # slab layout: per k, positives first then negatives (PERK is sorted);
    # each term owns W columns of TMPS.
    slab_off = {}
    off = 0
    for k in range(9):
        for t in range(len(_PERK[k])):
            slab_off[(k, t)] = off
            off += W
    SLABW = off  # 50 * W

    with TileContext(nc) as tc:
        with (
            tc.tile_pool(name="const", bufs=1) as cpool,
            tc.tile_pool(name="xin", bufs=4) as xpool,
            tc.tile_pool(name="hsb", bufs=2) as hpool,
            tc.tile_pool(name="tmp", bufs=2) as tpool,
            tc.tile_pool(name="acc", bufs=8) as apool,
            tc.tile_pool(name="ps", bufs=3, space="PSUM") as ppool,
            tc.tile_pool(name="po", bufs=1, space="PSUM") as popool,
        ):
            # constants: x2 transposed, R, +/- identity (cast on load)
            dma_c = nc.sync if mmdt == f32 else nc.gpsimd
            x2t_sb = cpool.tile([3, ns], mmdt, tag="x2t")
            dma_c.dma_start(out=x2t_sb[:], in_=x2t[:])
            r_sb = cpool.tile([3, NB * MUL], mmdt, tag="rmat")
            dma_c.dma_start(out=r_sb[:], in_=rmat[:])
            ident = cpool.tile([128, 128], mmdt, tag="ident")
            dma_c.dma_start(out=ident[:], in_=eye[:])
            nident = cpool.tile([128, 128], mmdt, tag="nident")
            nc.scalar.mul(nident[:], ident[:], -1.0)

            for (z0, P, q) in tiles:
                rows = q * P
                # X layout: [p, (q, i, u)]
                X = xpool.tile([128, zq * FREE], cdt, tag="X")
                dma_x = nc.sync if cdt == f32 else nc.gpsimd
                dma_x.dma_start(
                    out=X[:P, :q * FREE].rearrange("p (q c) -> p q c", c=FREE),
                    in_=x1[z0:z0 + rows, :].rearrange("(q p) c -> p q c", p=P),
                )
                # H layout: [p, (b, q, u)] so each block is zq*MUL contiguous
                H = hpool.tile([128, zq * HW], cdt, tag="H")
                H4 = H[:P, :].rearrange("p (b q u) -> p b q u", b=NB, q=zq)
                for qi in range(q):
                    lhsT = x2t_sb[:, z0 + qi * P: z0 + (qi + 1) * P]
                    for c0 in range(0, HW, 512):
                        c1 = min(c0 + 512, HW)
                        ps = ppool.tile([128, 512], f32, tag="ps")
                        nc.tensor.matmul(ps[:P, :c1 - c0], lhsT, r_sb[:, c0:c1],
                                         start=True, stop=True)
                        nc.scalar.copy(
                            out=H4[:, c0 // MUL:c1 // MUL, qi, :],
                            in_=ps[:P, :c1 - c0].rearrange(
                                "p (b u) -> p b u", u=MUL),
                        )

                # products into the slab (Vector/GpSimd)
                TMPS = tpool.tile([128, SLABW], cdt, tag="TMPS")
                X3 = X[:P, :q * FREE].rearrange("p (q c) -> p q c", c=FREE)
                for k in range(9):
                    eng = nc.vector if assign[k] == "v" else nc.gpsimd
                    for t, (ig, b, s) in enumerate(_PERK[k]):
                        o0 = slab_off[(k, t)]
                        dst = TMPS[:P, o0:o0 + q * MUL].rearrange(
                            "p (q u) -> p q u", u=MUL)
                        eng.tensor_mul(dst, X3[:, :, ig * MUL:(ig + 1) * MUL],
                                       H4[:, b, :q, :])

                # Tensor engine accumulates the slab into PSUM per k.
                # Same-sign term pairs fold into one matmul: the output AP
                # repeats the k-slab (step-0 dim), and PSUM's has_written
                # accumulate adds the two 256-col groups in a single pass.
                po = popool.tile([128, 9 * W], f32, tag="po")
                O = xpool.tile([128, 9 * W], f32, tag="O")
                for k in range(9):
                    ts = _PERK[k]
                    if k in EW_ACCUM_K:
                        # chain-add on the elementwise engine that made the
                        # products (relieves the Tensor engine); intermediate
                        # sums stay fp16 (2x mode) in fresh ping-pong tiles
                        # (in-place adds lose the 2x perf mode); the last add
                        # writes the fp32 output slab
                        eng = nc.vector if assign[k] == "v" else nc.gpsimd
                        ok = O[:P, k * W:k * W + q * MUL]
                        prev = TMPS[:P, slab_off[(k, 0)]:
                                    slab_off[(k, 0)] + q * MUL]
                        for t in range(1, len(ts)):
                            ot = slab_off[(k, t)]
                            tt = TMPS[:P, ot:ot + q * MUL]
                            if t == len(ts) - 1:
                                dst = ok
                            else:
                                A = apool.tile([128, zq * MUL], cdt, tag="acc")
                                dst = A[:P, :q * MUL]
                            if ts[t][2] > 0:
                                eng.tensor_add(dst, prev, tt)
                            else:
                                eng.tensor_sub(dst, prev, tt)
                            prev = dst
                        continue
                    dst1 = po[:P, k * W:k * W + q * MUL]
                    for t, (ig, b, s) in enumerate(ts):
                        o0 = slab_off[(k, t)]
                        idk = (ident if s > 0 else nident)[:P, :P]
                        nc.tensor.matmul(
                            dst1, idk, TMPS[:P, o0:o0 + q * MUL],
                            start=(t == 0), stop=(t == len(ts) - 1),
                            skip_group_check=True,
                        )
                # PSUM -> SBUF for the PE-accumulated k's, then HBM
                for k0, k1 in PE_COPY_RUNS:
                    nc.scalar.copy(out=O[:P, k0 * W:k1 * W],
                                   in_=po[:P, k0 * W:k1 * W])
                nc.sync.dma_start(
                    out=out[z0:z0 + rows, :].rearrange(
                        "(q p) (k u) -> p k q u", p=P, u=MUL),
                    in_=O[:P, :].rearrange("p (k q u) -> p k q u", k=9, q=zq)[
                        :, :, :q, :],
                )
    nc.compile()
    return nc


def _get_nc(ns, zq=2, cdt_name="float16"):
    key = (ns, zq, cdt_name)
    if key not in _CACHE:
        _CACHE[key] = _build_nc(ns, zq, cdt_name)
    return _CACHE[key]


def kernel(x1, x2, w):
    from concourse.bass_utils import run_bass_kernel_spmd

    N = x1.shape[0]
    assert N % N_CORES == 0
    ns = N // N_CORES
    R = _build_R(np.asarray(w, dtype=np.float32))
    x1f = np.ascontiguousarray(np.asarray(x1, dtype=np.float32).reshape(N, FREE))
    x2t = np.ascontiguousarray(np.asarray(x2, dtype=np.float32).reshape(N, 3).T)

    nc = _get_nc(ns)
    eye = np.eye(128, dtype=np.float32)
    in_maps = [
        {
            "x1": x1f[c * ns:(c + 1) * ns],
            "x2t": np.ascontiguousarray(x2t[:, c * ns:(c + 1) * ns]),
            "rmat": R,
            "eye": eye,
        }
        for c in range(N_CORES)
    ]
    res = run_bass_kernel_spmd(nc, in_maps, list(range(N_CORES)))
    out = np.concatenate([res.results[c]["out"] for c in range(N_CORES)], axis=0)
    return out.reshape(N, 9, MUL)


# revision 27
# speedup vs baseline: 1.0606x; 1.0606x over previous
"""Trainium2 Bass kernel for the e3nn-style uvu tensor product
(irreps 128x0e+128x1e+128x2e  x  1x1e, 6 paths, per-path u-weights).

Strategy (data-parallel over the batch axis N, 8 NeuronCores):
  out[z,k,u] = sum_t sign_t * x1[z, i_t, u] * H[z, b_t*128+u]
  H[z, :]    = x2row[z, :] @ R            (PE matmul, K=3, per 128-z tile)
  R[j, b*128+u] = |c_b| * w[p_b*128+u]    (host-built from w; b = (j,p,|c|) class)

Layout: z on SBUF partitions, (irrep k, channel u) along the free dim.
The 50 Wigner-3j terms become 50 tensor-tensor multiplies + 41 adds/subs,
split across the Vector and GpSimd engines; the Tensor engine builds H and
the Scalar engine moves H from PSUM to SBUF.
"""
import math
from math import factorial as _fact

import numpy as np

MUL = 128
PATHS = [(0, 1, 1), (1, 1, 0), (1, 1, 1), (1, 1, 2), (2, 1, 1), (2, 1, 2)]
SLBL = {0: 0, 1: 1, 2: 4}
N_CORES = 8
FREE = 9 * MUL  # 1152


# ---------------- Wigner 3j (identical math to the reference) ----------------
def _su2_cg(j1, j2, j3, m1, m2, m3):
    if m3 != m1 + m2:
        return 0.0
    vmin = max(-j1 + j2 + m3, -j1 + m1, 0)
    vmax = min(j2 + j3 + m1, j3 - j1 + j2, j3 + m3)
    f = _fact
    C = math.sqrt((2 * j3 + 1) * f(j3 + j1 - j2) * f(j3 - j1 + j2) * f(j1 + j2 - j3)
                  * f(j3 + m3) * f(j3 - m3)
                  / (f(j1 + j2 + j3 + 1) * f(j1 - m1) * f(j1 + m1) * f(j2 - m2) * f(j2 + m2)))
    S = 0.0
    for v in range(vmin, vmax + 1):
        S += (-1) ** (v + j2 + m2) * f(j2 + j3 + m1 - v) * f(j1 - m1 + v) \
             / (f(v) * f(j3 - j1 + j2 - v) * f(j3 + m3 - v) * f(v + j1 - j2 - m3))
    return C * S


def _change_basis_real_to_complex(l):
    q = np.zeros((2 * l + 1, 2 * l + 1), dtype=np.complex128)
    for m in range(-l, 0):
        q[l + m, l + abs(m)] = 1 / math.sqrt(2)
        q[l + m, l - abs(m)] = -1j / math.sqrt(2)
    q[l, l] = 1
    for m in range(1, l + 1):
        q[l + m, l + abs(m)] = (-1) ** m / math.sqrt(2)
        q[l + m, l - abs(m)] = 1j * (-1) ** m / math.sqrt(2)
    return (-1j) ** l * q


def _wigner_3j(l1, l2, l3):
    C = np.zeros((2 * l1 + 1, 2 * l2 + 1, 2 * l3 + 1))
    for m1 in range(-l1, l1 + 1):
        for m2 in range(-l2, l2 + 1):
            for m3 in range(-l3, l3 + 1):
                C[l1 + m1, l2 + m2, l3 + m3] = _su2_cg(l1, l2, l3, m1, m2, m3)
    Q1 = _change_basis_real_to_complex(l1)
    Q2 = _change_basis_real_to_complex(l2)
    Q3 = _change_basis_real_to_complex(l3)
    C = np.einsum('ij,kl,nm,ikn->jlm', Q1, Q2, Q3.conj(), C.astype(np.complex128))
    C = C.real
    return (C / np.linalg.norm(C)).astype(np.float32)


def _build_tables():
    terms = []
    for p, (l1, _l2, lo) in enumerate(PATHS):
        C = _wigner_3j(l1, 1, lo)
        for i in range(2 * l1 + 1):
            for j in range(3):
                for k in range(2 * lo + 1):
                    c = float(C[i, j, k])
                    if abs(c) > 1e-8:
                        terms.append((SLBL[lo] + k, SLBL[l1] + i, j, p, c))
    classes = sorted(set((j, p, round(abs(c), 7)) for (_, _, j, p, c) in terms))
    bid = {cl: i for i, cl in enumerate(classes)}
    perk = {k: [] for k in range(9)}
    for (kg, ig, j, p, c) in terms:
        perk[kg].append((ig, bid[(j, p, round(abs(c), 7))], 1 if c > 0 else -1))
    for k in range(9):
        perk[k].sort(key=lambda t: -t[2])
        assert perk[k][0][2] > 0
    return classes, perk


_CLASSES, _PERK = _build_tables()
NB = len(_CLASSES)  # 27


def _build_R(w):
    R = np.zeros((3, NB * MUL), dtype=np.float32)
    for b, (j, p, ac) in enumerate(_CLASSES):
        R[j, b * MUL:(b + 1) * MUL] = ac * w[p * MUL:(p + 1) * MUL]
    return R


# ---------------- Bass kernel ----------------
_CACHE = {}


def _plan_tiles(ns, zq):
    """[(z0, P, q)] tiles: q z-rows of P partitions each, z = z0 + qi*P + p."""
    tiles = []
    z0 = 0
    while ns - z0 >= zq * 128:
        tiles.append((z0, 128, zq))
        z0 += zq * 128
    while ns - z0 >= 128:
        tiles.append((z0, 128, 1))
        z0 += 128
    if ns - z0 > 0:
        tiles.append((z0, ns - z0, 1))
        z0 = ns
    return tiles


def _engine_split():
    """Hand-balanced per-k engine assignment (Vector ~2.5x GpSimd op rate)."""
    return {0: "v", 1: "v", 2: "v", 3: "v", 4: "g", 5: "v", 6: "v", 7: "v",
            8: "g"}


# k's whose accumulation runs as chain-adds on the elementwise engines
# (everything else accumulates on the Tensor engine via +/-I matmuls)
EW_ACCUM_K = {4, 5, 6, 8}
# packed PSUM slab index for PE-accumulated k's (EW k's never touch PSUM)
PO_IDX = {0: 0, 1: 1, 2: 2, 3: 3, 7: 4}


def _build_nc(ns, zq=3, cdt_name="float16"):
    """cdt_name: compute dtype for the elementwise term stage
    ("float16" ~5e-4 rel err, "bfloat16" ~4e-3, "float32" exact but slower).

    V3: the 50 term products go to a contiguous TMP slab (Vector/GpSimd);
    the per-k accumulation runs on the Tensor engine as +/-identity
    pass-through matmuls into PSUM (fp32), and the result is DMAed straight
    from PSUM to HBM.
    """
    import concourse.bacc as bacc
    import concourse.mybir as mybir
    from concourse.tile import TileContext

    f32 = mybir.dt.float32
    cdt = getattr(mybir.dt, cdt_name)
    mmdt = f32 if cdt == f32 else cdt  # matmul input dtype
    nc = bacc.Bacc("TRN2", target_bir_lowering=False, debug=False,
                   num_devices=N_CORES)
    x1 = nc.dram_tensor("x1", [ns, FREE], f32, kind="ExternalInput").ap()
    x2t = nc.dram_tensor("x2t", [3, ns], f32, kind="ExternalInput").ap()
    rmat = nc.dram_tensor("rmat", [3, NB * MUL], f32, kind="ExternalInput").ap()
    eye = nc.dram_tensor("eye", [128, 128], f32, kind="ExternalInput").ap()
    out = nc.dram_tensor("out", [ns, FREE], f32, kind="ExternalOutput").ap()

    tiles = _plan_tiles(ns, zq)
    assign = _engine_split()
    HW = NB * MUL  # H width per z-row: 3456
    W = zq * MUL   # free width of one (k) slab / one product

    # slab layout: per k, positives first then negatives (PERK is sorted);
    # each term owns W columns of TMPS.
    slab_off = {}
    off = 0
    for k in range(9):
        for t in range(len(_PERK[k])):
            slab_off[(k, t)] = off
            off += W
    SLABW = off  # 50 * W

    with TileContext(nc) as tc:
        with (
            tc.tile_pool(name="const", bufs=1) as cpool,
            tc.tile_pool(name="xin", bufs=3) as xpool,
            tc.tile_pool(name="out", bufs=2) as opool,
            tc.tile_pool(name="hsb", bufs=2) as hpool,
            tc.tile_pool(name="tmp", bufs=2) as tpool,
            tc.tile_pool(name="acc", bufs=6) as apool,
            tc.tile_pool(name="ps", bufs=3, space="PSUM") as ppool,
            tc.tile_pool(name="po", bufs=1, space="PSUM") as popool,
        ):
            # constants: x2 transposed, R, +/- identity (cast on load)
            dma_c = nc.sync if mmdt == f32 else nc.gpsimd
            x2t_sb = cpool.tile([3, ns], mmdt, tag="x2t")
            dma_c.dma_start(out=x2t_sb[:], in_=x2t[:])
            r_sb = cpool.tile([3, NB * MUL], mmdt, tag="rmat")
            dma_c.dma_start(out=r_sb[:], in_=rmat[:])
            ident = cpool.tile([128, 128], mmdt, tag="ident")
            dma_c.dma_start(out=ident[:], in_=eye[:])
            nident = cpool.tile([128, 128], mmdt, tag="nident")
            nc.scalar.mul(nident[:], ident[:], -1.0)

            for (z0, P, q) in tiles:
                rows = q * P
                # X layout: [p, (q, i, u)]
                X = xpool.tile([128, zq * FREE], cdt, tag="X")
                dma_x = nc.sync if cdt == f32 else nc.gpsimd
                dma_x.dma_start(
                    out=X[:P, :q * FREE].rearrange("p (q c) -> p q c", c=FREE),
                    in_=x1[z0:z0 + rows, :].rearrange("(q p) c -> p q c", p=P),
                )
                # H layout: [p, (b, q, u)] so each block is zq*MUL contiguous
                H = hpool.tile([128, zq * HW], cdt, tag="H")
                H4 = H[:P, :].rearrange("p (b q u) -> p b q u", b=NB, q=zq)
                for qi in range(q):
                    lhsT = x2t_sb[:, z0 + qi * P: z0 + (qi + 1) * P]
                    for c0 in range(0, HW, 512):
                        c1 = min(c0 + 512, HW)
                        ps = ppool.tile([128, 512], f32, tag="ps")
                        nc.tensor.matmul(ps[:P, :c1 - c0], lhsT, r_sb[:, c0:c1],
                                         start=True, stop=True)
                        nc.scalar.copy(
                            out=H4[:, c0 // MUL:c1 // MUL, qi, :],
                            in_=ps[:P, :c1 - c0].rearrange(
                                "p (b u) -> p b u", u=MUL),
                        )

                # products into the slab (Vector/GpSimd)
                TMPS = tpool.tile([128, SLABW], cdt, tag="TMPS")
                X3 = X[:P, :q * FREE].rearrange("p (q c) -> p q c", c=FREE)
                for k in range(9):
                    eng = nc.vector if assign[k] == "v" else nc.gpsimd
                    for t, (ig, b, s) in enumerate(_PERK[k]):
                        o0 = slab_off[(k, t)]
                        dst = TMPS[:P, o0:o0 + q * MUL].rearrange(
                            "p (q u) -> p q u", u=MUL)
                        eng.tensor_mul(dst, X3[:, :, ig * MUL:(ig + 1) * MUL],
                                       H4[:, b, :q, :])

                # Tensor engine accumulates the slab into PSUM per k.
                # Same-sign term pairs fold into one matmul: the output AP
                # repeats the k-slab (step-0 dim), and PSUM's has_written
                # accumulate adds the two 256-col groups in a single pass.
                po = popool.tile([128, 5 * W], f32, tag="po")
                O = opool.tile([128, 9 * W], cdt, tag="O")
                for k in range(9):
                    ts = _PERK[k]
                    if k in EW_ACCUM_K:
                        # chain-add on the elementwise engine that made the
                        # products (relieves the Tensor engine); intermediate
                        # sums stay fp16 (2x mode) in fresh ping-pong tiles
                        # (in-place adds lose the 2x perf mode); the last add
                        # writes the fp32 output slab
                        eng = nc.vector if assign[k] == "v" else nc.gpsimd
                        ok = O[:P, k * W:k * W + q * MUL]
                        prev = TMPS[:P, slab_off[(k, 0)]:
                                    slab_off[(k, 0)] + q * MUL]
                        for t in range(1, len(ts)):
                            ot = slab_off[(k, t)]
                            tt = TMPS[:P, ot:ot + q * MUL]
                            if t == len(ts) - 1:
                                dst = ok
                            else:
                                A = apool.tile([128, zq * MUL], cdt, tag="acc")
                                dst = A[:P, :q * MUL]
                            if ts[t][2] > 0:
                                eng.tensor_add(dst, prev, tt)
                            else:
                                eng.tensor_sub(dst, prev, tt)
                            prev = dst
                        continue
                    dst1 = po[:P, PO_IDX[k] * W:PO_IDX[k] * W + q * MUL]
                    for t, (ig, b, s) in enumerate(ts):
                        o0 = slab_off[(k, t)]
                        idk = (ident if s > 0 else nident)[:P, :P]
                        nc.tensor.matmul(
                            dst1, idk, TMPS[:P, o0:o0 + q * MUL],
                            start=(t == 0), stop=(t == len(ts) - 1),
                            skip_group_check=True,
                        )
                # PSUM -> SBUF (cast to fp16) for the PE-accumulated k's
                nc.scalar.copy(out=O[:P, 0:4 * W], in_=po[:P, 0:4 * W])
                nc.scalar.copy(out=O[:P, 7 * W:8 * W], in_=po[:P, 4 * W:5 * W])
                nc.gpsimd.dma_start(
                    out=out[z0:z0 + rows, :].rearrange(
                        "(q p) (k u) -> p k q u", p=P, u=MUL),
                    in_=O[:P, :].rearrange("p (k q u) -> p k q u", k=9, q=zq)[
                        :, :, :q, :],
                )
    nc.compile()
    return nc


def _get_nc(ns, zq=3, cdt_name="float16"):
    key = (ns, zq, cdt_name)
    if key not in _CACHE:
        _CACHE[key] = _build_nc(ns, zq, cdt_name)
    return _CACHE[key]


def kernel(x1, x2, w):
    from concourse.bass_utils import run_bass_kernel_spmd

    N = x1.shape[0]
    assert N % N_CORES == 0
    ns = N // N_CORES
    R = _build_R(np.asarray(w, dtype=np.float32))
    x1f = np.ascontiguousarray(np.asarray(x1, dtype=np.float32).reshape(N, FREE))
    x2t = np.ascontiguousarray(np.asarray(x2, dtype=np.float32).reshape(N, 3).T)

    nc = _get_nc(ns)
    eye = np.eye(128, dtype=np.float32)
    in_maps = [
        {
            "x1": x1f[c * ns:(c + 1) * ns],
            "x2t": np.ascontiguousarray(x2t[:, c * ns:(c + 1) * ns]),
            "rmat": R,
            "eye": eye,
        }
        for c in range(N_CORES)
    ]
    res = run_bass_kernel_spmd(nc, in_maps, list(range(N_CORES)))
    out = np.concatenate([res.results[c]["out"] for c in range(N_CORES)], axis=0)
    return out.reshape(N, 9, MUL)


# revision 28
# speedup vs baseline: 1.1397x; 1.0745x over previous
"""Trainium2 Bass kernel for the e3nn-style uvu tensor product
(irreps 128x0e+128x1e+128x2e  x  1x1e, 6 paths, per-path u-weights).

Strategy (data-parallel over the batch axis N, 8 NeuronCores):
  out[z,k,u] = sum_t sign_t * x1[z, i_t, u] * H[z, b_t*128+u]
  H[z, :]    = x2row[z, :] @ R            (PE matmul, K=3, per 128-z tile)
  R[j, b*128+u] = |c_b| * w[p_b*128+u]    (host-built from w; b = (j,p,|c|) class)

Layout: z on SBUF partitions, (irrep k, channel u) along the free dim.
The 50 Wigner-3j terms become 50 tensor-tensor multiplies + 41 adds/subs,
split across the Vector and GpSimd engines; the Tensor engine builds H and
the Scalar engine moves H from PSUM to SBUF.
"""
import math
from math import factorial as _fact

import numpy as np

MUL = 128
PATHS = [(0, 1, 1), (1, 1, 0), (1, 1, 1), (1, 1, 2), (2, 1, 1), (2, 1, 2)]
SLBL = {0: 0, 1: 1, 2: 4}
N_CORES = 8
FREE = 9 * MUL  # 1152


# ---------------- Wigner 3j (identical math to the reference) ----------------
def _su2_cg(j1, j2, j3, m1, m2, m3):
    if m3 != m1 + m2:
        return 0.0
    vmin = max(-j1 + j2 + m3, -j1 + m1, 0)
    vmax = min(j2 + j3 + m1, j3 - j1 + j2, j3 + m3)
    f = _fact
    C = math.sqrt((2 * j3 + 1) * f(j3 + j1 - j2) * f(j3 - j1 + j2) * f(j1 + j2 - j3)
                  * f(j3 + m3) * f(j3 - m3)
                  / (f(j1 + j2 + j3 + 1) * f(j1 - m1) * f(j1 + m1) * f(j2 - m2) * f(j2 + m2)))
    S = 0.0
    for v in range(vmin, vmax + 1):
        S += (-1) ** (v + j2 + m2) * f(j2 + j3 + m1 - v) * f(j1 - m1 + v) \
             / (f(v) * f(j3 - j1 + j2 - v) * f(j3 + m3 - v) * f(v + j1 - j2 - m3))
    return C * S


def _change_basis_real_to_complex(l):
    q = np.zeros((2 * l + 1, 2 * l + 1), dtype=np.complex128)
    for m in range(-l, 0):
        q[l + m, l + abs(m)] = 1 / math.sqrt(2)
        q[l + m, l - abs(m)] = -1j / math.sqrt(2)
    q[l, l] = 1
    for m in range(1, l + 1):
        q[l + m, l + abs(m)] = (-1) ** m / math.sqrt(2)
        q[l + m, l - abs(m)] = 1j * (-1) ** m / math.sqrt(2)
    return (-1j) ** l * q


def _wigner_3j(l1, l2, l3):
    C = np.zeros((2 * l1 + 1, 2 * l2 + 1, 2 * l3 + 1))
    for m1 in range(-l1, l1 + 1):
        for m2 in range(-l2, l2 + 1):
            for m3 in range(-l3, l3 + 1):
                C[l1 + m1, l2 + m2, l3 + m3] = _su2_cg(l1, l2, l3, m1, m2, m3)
    Q1 = _change_basis_real_to_complex(l1)
    Q2 = _change_basis_real_to_complex(l2)
    Q3 = _change_basis_real_to_complex(l3)
    C = np.einsum('ij,kl,nm,ikn->jlm', Q1, Q2, Q3.conj(), C.astype(np.complex128))
    C = C.real
    return (C / np.linalg.norm(C)).astype(np.float32)


def _build_tables():
    terms = []
    for p, (l1, _l2, lo) in enumerate(PATHS):
        C = _wigner_3j(l1, 1, lo)
        for i in range(2 * l1 + 1):
            for j in range(3):
                for k in range(2 * lo + 1):
                    c = float(C[i, j, k])
                    if abs(c) > 1e-8:
                        terms.append((SLBL[lo] + k, SLBL[l1] + i, j, p, c))
    classes = sorted(set((j, p, round(abs(c), 7)) for (_, _, j, p, c) in terms))
    bid = {cl: i for i, cl in enumerate(classes)}
    perk = {k: [] for k in range(9)}
    for (kg, ig, j, p, c) in terms:
        perk[kg].append((ig, bid[(j, p, round(abs(c), 7))], 1 if c > 0 else -1))
    for k in range(9):
        perk[k].sort(key=lambda t: -t[2])
        assert perk[k][0][2] > 0
    return classes, perk


_CLASSES, _PERK = _build_tables()
NB = len(_CLASSES)  # 27


def _build_R(w):
    R = np.zeros((3, NB * MUL), dtype=np.float32)
    for b, (j, p, ac) in enumerate(_CLASSES):
        R[j, b * MUL:(b + 1) * MUL] = ac * w[p * MUL:(p + 1) * MUL]
    return R


# ---------------- Bass kernel ----------------
_CACHE = {}


def _plan_tiles(ns, zq):
    """[(z0, P, q)] tiles: q z-rows of P partitions each, z = z0 + qi*P + p."""
    tiles = []
    z0 = 0
    while ns - z0 >= zq * 128:
        tiles.append((z0, 128, zq))
        z0 += zq * 128
    while ns - z0 >= 128:
        tiles.append((z0, 128, 1))
        z0 += 128
    if ns - z0 > 0:
        tiles.append((z0, ns - z0, 1))
        z0 = ns
    return tiles


def _engine_split():
    """Hand-balanced per-k engine assignment (Vector ~2.5x GpSimd op rate)."""
    return {0: "v", 1: "v", 2: "v", 3: "v", 4: "g", 5: "v", 6: "v", 7: "v",
            8: "g"}


# k's whose accumulation runs as chain-adds on the elementwise engines
# (everything else accumulates on the Tensor engine via +/-I matmuls)
EW_ACCUM_K = {4, 5, 6, 8}
# packed PSUM slab index for PE-accumulated k's (EW k's never touch PSUM)
PO_IDX = {0: 0, 1: 1, 2: 2, 3: 3, 7: 4}


def _build_nc(ns, zq=3, cdt_name="float16"):
    """cdt_name: compute dtype for the elementwise term stage
    ("float16" ~5e-4 rel err, "bfloat16" ~4e-3, "float32" exact but slower).

    V3: the 50 term products go to a contiguous TMP slab (Vector/GpSimd);
    the per-k accumulation runs on the Tensor engine as +/-identity
    pass-through matmuls into PSUM (fp32), and the result is DMAed straight
    from PSUM to HBM.
    """
    import concourse.bacc as bacc
    import concourse.mybir as mybir
    from concourse.tile import TileContext

    f32 = mybir.dt.float32
    cdt = getattr(mybir.dt, cdt_name)
    mmdt = f32 if cdt == f32 else cdt  # matmul input dtype
    nc = bacc.Bacc("TRN2", target_bir_lowering=False, debug=False,
                   num_devices=N_CORES)
    x1 = nc.dram_tensor("x1", [ns, FREE], f32, kind="ExternalInput").ap()
    x2t = nc.dram_tensor("x2t", [3, ns], f32, kind="ExternalInput").ap()
    rmat = nc.dram_tensor("rmat", [3, NB * MUL], f32, kind="ExternalInput").ap()
    eye = nc.dram_tensor("eye", [128, 128], f32, kind="ExternalInput").ap()
    out = nc.dram_tensor("out", [ns, FREE], f32, kind="ExternalOutput").ap()

    tiles = _plan_tiles(ns, zq)
    assign = _engine_split()
    HW = NB * MUL  # H width per z-row: 3456
    W = zq * MUL   # free width of one (k) slab / one product

    # slab layout: per k, positives first then negatives (PERK is sorted);
    # each term owns W columns of TMPS.
    slab_off = {}
    off = 0
    for k in range(9):
        for t in range(len(_PERK[k])):
            slab_off[(k, t)] = off
            off += W
    SLABW = off  # 50 * W

    with TileContext(nc) as tc:
        with (
            tc.tile_pool(name="const", bufs=1) as cpool,
            tc.tile_pool(name="xin", bufs=3) as xpool,
            tc.tile_pool(name="out", bufs=2) as opool,
            tc.tile_pool(name="hsb", bufs=2) as hpool,
            tc.tile_pool(name="tmp", bufs=2) as tpool,
            tc.tile_pool(name="acc", bufs=6) as apool,
            tc.tile_pool(name="ps", bufs=3, space="PSUM") as ppool,
            tc.tile_pool(name="po", bufs=1, space="PSUM") as popool,
        ):
            # constants: x2 transposed, R, +/- identity (cast on load)
            dma_c = nc.sync if mmdt == f32 else nc.gpsimd
            x2t_sb = cpool.tile([3, ns], mmdt, tag="x2t")
            dma_c.dma_start(out=x2t_sb[:], in_=x2t[:])
            r_sb = cpool.tile([3, NB * MUL], mmdt, tag="rmat")
            dma_c.dma_start(out=r_sb[:], in_=rmat[:])
            ident = cpool.tile([128, 128], mmdt, tag="ident")
            dma_c.dma_start(out=ident[:], in_=eye[:])
            nident = cpool.tile([128, 128], mmdt, tag="nident")
            nc.scalar.mul(nident[:], ident[:], -1.0)

            for (z0, P, q) in tiles:
                rows = q * P
                # X layout: [p, (q, i, u)]
                X = xpool.tile([128, zq * FREE], cdt, tag="X")
                dma_x = nc.sync if cdt == f32 else nc.gpsimd
                dma_x.dma_start(
                    out=X[:P, :q * FREE].rearrange("p (q c) -> p q c", c=FREE),
                    in_=x1[z0:z0 + rows, :].rearrange("(q p) c -> p q c", p=P),
                )
                # H layout: [p, (b, q, u)] so each block is zq*MUL contiguous
                H = hpool.tile([128, zq * HW], cdt, tag="H")
                H4 = H[:P, :].rearrange("p (b q u) -> p b q u", b=NB, q=zq)
                for qi in range(q):
                    lhsT = x2t_sb[:, z0 + qi * P: z0 + (qi + 1) * P]
                    for c0 in range(0, HW, 512):
                        c1 = min(c0 + 512, HW)
                        ps = ppool.tile([128, 512], f32, tag="ps")
                        nc.tensor.matmul(ps[:P, :c1 - c0], lhsT, r_sb[:, c0:c1],
                                         start=True, stop=True)
                        nc.scalar.copy(
                            out=H4[:, c0 // MUL:c1 // MUL, qi, :],
                            in_=ps[:P, :c1 - c0].rearrange(
                                "p (b u) -> p b u", u=MUL),
                        )

                # products into the slab (Vector/GpSimd)
                TMPS = tpool.tile([128, SLABW], cdt, tag="TMPS")
                X3 = X[:P, :q * FREE].rearrange("p (q c) -> p q c", c=FREE)
                for k in range(9):
                    eng = nc.vector if assign[k] == "v" else nc.gpsimd
                    for t, (ig, b, s) in enumerate(_PERK[k]):
                        o0 = slab_off[(k, t)]
                        dst = TMPS[:P, o0:o0 + q * MUL].rearrange(
                            "p (q u) -> p q u", u=MUL)
                        eng.tensor_mul(dst, X3[:, :, ig * MUL:(ig + 1) * MUL],
                                       H4[:, b, :q, :])

                # Tensor engine accumulates the slab into PSUM per k.
                # Same-sign term pairs fold into one matmul: the output AP
                # repeats the k-slab (step-0 dim), and PSUM's has_written
                # accumulate adds the two 256-col groups in a single pass.
                po = popool.tile([128, 5 * W], f32, tag="po")
                O = opool.tile([128, 9 * W], cdt, tag="O")
                for k in range(9):
                    ts = _PERK[k]
                    if k in EW_ACCUM_K:
                        # chain-add on the elementwise engine that made the
                        # products (relieves the Tensor engine); intermediate
                        # sums stay fp16 (2x mode) in fresh ping-pong tiles
                        # (in-place adds lose the 2x perf mode); the last add
                        # writes the fp32 output slab
                        eng = nc.vector if k in (5, 6, 8) else nc.gpsimd
                        ok = O[:P, k * W:k * W + q * MUL]
                        prev = TMPS[:P, slab_off[(k, 0)]:
                                    slab_off[(k, 0)] + q * MUL]
                        for t in range(1, len(ts)):
                            ot = slab_off[(k, t)]
                            tt = TMPS[:P, ot:ot + q * MUL]
                            if t == len(ts) - 1:
                                dst = ok
                            else:
                                A = apool.tile([128, zq * MUL], cdt, tag="acc")
                                dst = A[:P, :q * MUL]
                            if ts[t][2] > 0:
                                eng.tensor_add(dst, prev, tt)
                            else:
                                eng.tensor_sub(dst, prev, tt)
                            prev = dst
                        continue
                    dst1 = po[:P, PO_IDX[k] * W:PO_IDX[k] * W + q * MUL]
                    for t, (ig, b, s) in enumerate(ts):
                        o0 = slab_off[(k, t)]
                        idk = (ident if s > 0 else nident)[:P, :P]
                        nc.tensor.matmul(
                            dst1, idk, TMPS[:P, o0:o0 + q * MUL],
                            start=(t == 0), stop=(t == len(ts) - 1),
                            skip_group_check=True,
                        )
                # PSUM -> SBUF (cast to fp16) for the PE-accumulated k's
                nc.scalar.copy(out=O[:P, 0:4 * W], in_=po[:P, 0:4 * W])
                nc.scalar.copy(out=O[:P, 7 * W:8 * W], in_=po[:P, 4 * W:5 * W])
                nc.gpsimd.dma_start(
                    out=out[z0:z0 + rows, :].rearrange(
                        "(q p) (k u) -> p k q u", p=P, u=MUL),
                    in_=O[:P, :].rearrange("p (k q u) -> p k q u", k=9, q=zq)[
                        :, :, :q, :],
                )
    nc.compile()
    return nc


def _get_nc(ns, zq=3, cdt_name="float16"):
    key = (ns, zq, cdt_name)
    if key not in _CACHE:
        _CACHE[key] = _build_nc(ns, zq, cdt_name)
    return _CACHE[key]


def kernel(x1, x2, w):
    from concourse.bass_utils import run_bass_kernel_spmd

    N = x1.shape[0]
    assert N % N_CORES == 0
    ns = N // N_CORES
    R = _build_R(np.asarray(w, dtype=np.float32))
    x1f = np.ascontiguousarray(np.asarray(x1, dtype=np.float32).reshape(N, FREE))
    x2t = np.ascontiguousarray(np.asarray(x2, dtype=np.float32).reshape(N, 3).T)

    nc = _get_nc(ns)
    eye = np.eye(128, dtype=np.float32)
    in_maps = [
        {
            "x1": x1f[c * ns:(c + 1) * ns],
            "x2t": np.ascontiguousarray(x2t[:, c * ns:(c + 1) * ns]),
            "rmat": R,
            "eye": eye,
        }
        for c in range(N_CORES)
    ]
    res = run_bass_kernel_spmd(nc, in_maps, list(range(N_CORES)))
    out = np.concatenate([res.results[c]["out"] for c in range(N_CORES)], axis=0)
    return out.reshape(N, 9, MUL)


# revision 29
# speedup vs baseline: 1.2505x; 1.0973x over previous
"""Trainium2 Bass kernel for the e3nn-style uvu tensor product
(irreps 128x0e+128x1e+128x2e  x  1x1e, 6 paths, per-path u-weights).

Strategy (data-parallel over the batch axis N, 8 NeuronCores):
  out[z,k,u] = sum_t sign_t * x1[z, i_t, u] * H[z, b_t*128+u]
  H[z, :]    = x2row[z, :] @ R            (PE matmul, K=3, per 128-z tile)
  R[j, b*128+u] = |c_b| * w[p_b*128+u]    (host-built from w; b = (j,p,|c|) class)

Layout: z on SBUF partitions, (irrep k, channel u) along the free dim.
The 50 Wigner-3j terms become 50 tensor-tensor multiplies + 41 adds/subs,
split across the Vector and GpSimd engines; the Tensor engine builds H and
the Scalar engine moves H from PSUM to SBUF.
"""
import math
from math import factorial as _fact

import numpy as np

MUL = 128
PATHS = [(0, 1, 1), (1, 1, 0), (1, 1, 1), (1, 1, 2), (2, 1, 1), (2, 1, 2)]
SLBL = {0: 0, 1: 1, 2: 4}
N_CORES = 8
FREE = 9 * MUL  # 1152


# ---------------- Wigner 3j (identical math to the reference) ----------------
def _su2_cg(j1, j2, j3, m1, m2, m3):
    if m3 != m1 + m2:
        return 0.0
    vmin = max(-j1 + j2 + m3, -j1 + m1, 0)
    vmax = min(j2 + j3 + m1, j3 - j1 + j2, j3 + m3)
    f = _fact
    C = math.sqrt((2 * j3 + 1) * f(j3 + j1 - j2) * f(j3 - j1 + j2) * f(j1 + j2 - j3)
                  * f(j3 + m3) * f(j3 - m3)
                  / (f(j1 + j2 + j3 + 1) * f(j1 - m1) * f(j1 + m1) * f(j2 - m2) * f(j2 + m2)))
    S = 0.0
    for v in range(vmin, vmax + 1):
        S += (-1) ** (v + j2 + m2) * f(j2 + j3 + m1 - v) * f(j1 - m1 + v) \
             / (f(v) * f(j3 - j1 + j2 - v) * f(j3 + m3 - v) * f(v + j1 - j2 - m3))
    return C * S


def _change_basis_real_to_complex(l):
    q = np.zeros((2 * l + 1, 2 * l + 1), dtype=np.complex128)
    for m in range(-l, 0):
        q[l + m, l + abs(m)] = 1 / math.sqrt(2)
        q[l + m, l - abs(m)] = -1j / math.sqrt(2)
    q[l, l] = 1
    for m in range(1, l + 1):
        q[l + m, l + abs(m)] = (-1) ** m / math.sqrt(2)
        q[l + m, l - abs(m)] = 1j * (-1) ** m / math.sqrt(2)
    return (-1j) ** l * q


def _wigner_3j(l1, l2, l3):
    C = np.zeros((2 * l1 + 1, 2 * l2 + 1, 2 * l3 + 1))
    for m1 in range(-l1, l1 + 1):
        for m2 in range(-l2, l2 + 1):
            for m3 in range(-l3, l3 + 1):
                C[l1 + m1, l2 + m2, l3 + m3] = _su2_cg(l1, l2, l3, m1, m2, m3)
    Q1 = _change_basis_real_to_complex(l1)
    Q2 = _change_basis_real_to_complex(l2)
    Q3 = _change_basis_real_to_complex(l3)
    C = np.einsum('ij,kl,nm,ikn->jlm', Q1, Q2, Q3.conj(), C.astype(np.complex128))
    C = C.real
    return (C / np.linalg.norm(C)).astype(np.float32)


def _build_tables():
    terms = []
    for p, (l1, _l2, lo) in enumerate(PATHS):
        C = _wigner_3j(l1, 1, lo)
        for i in range(2 * l1 + 1):
            for j in range(3):
                for k in range(2 * lo + 1):
                    c = float(C[i, j, k])
                    if abs(c) > 1e-8:
                        terms.append((SLBL[lo] + k, SLBL[l1] + i, j, p, c))
    classes = sorted(set((j, p, round(abs(c), 7)) for (_, _, j, p, c) in terms))
    bid = {cl: i for i, cl in enumerate(classes)}
    perk = {k: [] for k in range(9)}
    for (kg, ig, j, p, c) in terms:
        perk[kg].append((ig, bid[(j, p, round(abs(c), 7))], 1 if c > 0 else -1))
    for k in range(9):
        perk[k].sort(key=lambda t: -t[2])
        assert perk[k][0][2] > 0
    return classes, perk


_CLASSES, _PERK = _build_tables()
NB = len(_CLASSES)  # 27


def _build_R(w):
    R = np.zeros((3, NB * MUL), dtype=np.float32)
    for b, (j, p, ac) in enumerate(_CLASSES):
        R[j, b * MUL:(b + 1) * MUL] = ac * w[p * MUL:(p + 1) * MUL]
    return R


# ---------------- Bass kernel ----------------
_CACHE = {}


def _plan_tiles(ns, zq):
    """[(z0, P, q)] tiles: q z-rows of P partitions each, z = z0 + qi*P + p."""
    tiles = []
    z0 = 0
    while ns - z0 >= zq * 128:
        tiles.append((z0, 128, zq))
        z0 += zq * 128
    while ns - z0 >= 128:
        tiles.append((z0, 128, 1))
        z0 += 128
    if ns - z0 > 0:
        tiles.append((z0, ns - z0, 1))
        z0 = ns
    return tiles


def _engine_split():
    """Hand-balanced per-k engine assignment (Vector ~2.5x GpSimd op rate)."""
    return {0: "v", 1: "v", 2: "v", 3: "v", 4: "g", 5: "v", 6: "v", 7: "v",
            8: "g"}


# k's whose accumulation runs as chain-adds on the elementwise engines
# (everything else accumulates on the Tensor engine via +/-I matmuls)
EW_ACCUM_K = {0, 4, 5, 6, 8}
# packed PSUM slab index for PE-accumulated k's (EW k's never touch PSUM)
PO_IDX = {1: 0, 2: 1, 3: 2, 7: 3}


def _build_nc(ns, zq=3, cdt_name="float16"):
    """cdt_name: compute dtype for the elementwise term stage
    ("float16" ~5e-4 rel err, "bfloat16" ~4e-3, "float32" exact but slower).

    V3: the 50 term products go to a contiguous TMP slab (Vector/GpSimd);
    the per-k accumulation runs on the Tensor engine as +/-identity
    pass-through matmuls into PSUM (fp32), and the result is DMAed straight
    from PSUM to HBM.
    """
    import concourse.bacc as bacc
    import concourse.mybir as mybir
    from concourse.tile import TileContext

    f32 = mybir.dt.float32
    cdt = getattr(mybir.dt, cdt_name)
    mmdt = f32 if cdt == f32 else cdt  # matmul input dtype
    nc = bacc.Bacc("TRN2", target_bir_lowering=False, debug=False,
                   num_devices=N_CORES)
    x1 = nc.dram_tensor("x1", [ns, FREE], f32, kind="ExternalInput").ap()
    x2t = nc.dram_tensor("x2t", [3, ns], f32, kind="ExternalInput").ap()
    rmat = nc.dram_tensor("rmat", [3, NB * MUL], f32, kind="ExternalInput").ap()
    eye = nc.dram_tensor("eye", [128, 128], f32, kind="ExternalInput").ap()
    out = nc.dram_tensor("out", [ns, FREE], f32, kind="ExternalOutput").ap()

    tiles = _plan_tiles(ns, zq)
    assign = _engine_split()
    HW = NB * MUL  # H width per z-row: 3456
    W = zq * MUL   # free width of one (k) slab / one product

    # slab layout: per k, positives first then negatives (PERK is sorted);
    # each term owns W columns of TMPS.
    slab_off = {}
    off = 0
    for k in range(9):
        for t in range(len(_PERK[k])):
            slab_off[(k, t)] = off
            off += W
    SLABW = off  # 50 * W

    with TileContext(nc) as tc:
        with (
            tc.tile_pool(name="const", bufs=1) as cpool,
            tc.tile_pool(name="xin", bufs=3) as xpool,
            tc.tile_pool(name="out", bufs=2) as opool,
            tc.tile_pool(name="hsb", bufs=2) as hpool,
            tc.tile_pool(name="tmp", bufs=2) as tpool,
            tc.tile_pool(name="acc", bufs=6) as apool,
            tc.tile_pool(name="ps", bufs=3, space="PSUM") as ppool,
            tc.tile_pool(name="po", bufs=1, space="PSUM") as popool,
        ):
            # constants: x2 transposed, R, +/- identity (cast on load)
            dma_c = nc.sync if mmdt == f32 else nc.gpsimd
            x2t_sb = cpool.tile([3, ns], mmdt, tag="x2t")
            dma_c.dma_start(out=x2t_sb[:], in_=x2t[:])
            r_sb = cpool.tile([3, NB * MUL], mmdt, tag="rmat")
            dma_c.dma_start(out=r_sb[:], in_=rmat[:])
            ident = cpool.tile([128, 128], mmdt, tag="ident")
            dma_c.dma_start(out=ident[:], in_=eye[:])
            nident = cpool.tile([128, 128], mmdt, tag="nident")
            nc.scalar.mul(nident[:], ident[:], -1.0)

            for (z0, P, q) in tiles:
                rows = q * P
                # X layout: [p, (q, i, u)]
                X = xpool.tile([128, zq * FREE], cdt, tag="X")
                dma_x = nc.sync if cdt == f32 else nc.gpsimd
                dma_x.dma_start(
                    out=X[:P, :q * FREE].rearrange("p (q c) -> p q c", c=FREE),
                    in_=x1[z0:z0 + rows, :].rearrange("(q p) c -> p q c", p=P),
                )
                # H layout: [p, (b, q, u)] so each block is zq*MUL contiguous
                H = hpool.tile([128, zq * HW], cdt, tag="H")
                H4 = H[:P, :].rearrange("p (b q u) -> p b q u", b=NB, q=zq)
                for qi in range(q):
                    lhsT = x2t_sb[:, z0 + qi * P: z0 + (qi + 1) * P]
                    for c0 in range(0, HW, 512):
                        c1 = min(c0 + 512, HW)
                        ps = ppool.tile([128, 512], f32, tag="ps")
                        nc.tensor.matmul(ps[:P, :c1 - c0], lhsT, r_sb[:, c0:c1],
                                         start=True, stop=True)
                        nc.scalar.copy(
                            out=H4[:, c0 // MUL:c1 // MUL, qi, :],
                            in_=ps[:P, :c1 - c0].rearrange(
                                "p (b u) -> p b u", u=MUL),
                        )

                # products into the slab (Vector/GpSimd)
                TMPS = tpool.tile([128, SLABW], cdt, tag="TMPS")
                X3 = X[:P, :q * FREE].rearrange("p (q c) -> p q c", c=FREE)
                for k in range(9):
                    eng = nc.vector if assign[k] == "v" else nc.gpsimd
                    for t, (ig, b, s) in enumerate(_PERK[k]):
                        o0 = slab_off[(k, t)]
                        dst = TMPS[:P, o0:o0 + q * MUL].rearrange(
                            "p (q u) -> p q u", u=MUL)
                        eng.tensor_mul(dst, X3[:, :, ig * MUL:(ig + 1) * MUL],
                                       H4[:, b, :q, :])

                # Tensor engine accumulates the slab into PSUM per k.
                # Same-sign term pairs fold into one matmul: the output AP
                # repeats the k-slab (step-0 dim), and PSUM's has_written
                # accumulate adds the two 256-col groups in a single pass.
                po = popool.tile([128, 4 * W], f32, tag="po")
                O = opool.tile([128, 9 * W], cdt, tag="O")
                for k in range(9):
                    ts = _PERK[k]
                    if k in EW_ACCUM_K:
                        # chain-add on the elementwise engine that made the
                        # products (relieves the Tensor engine); intermediate
                        # sums stay fp16 (2x mode) in fresh ping-pong tiles
                        # (in-place adds lose the 2x perf mode); the last add
                        # writes the fp32 output slab
                        eng = nc.vector if k in (0, 4, 5, 6, 8) else nc.gpsimd
                        ok = O[:P, k * W:k * W + q * MUL]
                        prev = TMPS[:P, slab_off[(k, 0)]:
                                    slab_off[(k, 0)] + q * MUL]
                        for t in range(1, len(ts)):
                            ot = slab_off[(k, t)]
                            tt = TMPS[:P, ot:ot + q * MUL]
                            if t == len(ts) - 1:
                                dst = ok
                            else:
                                A = apool.tile([128, zq * MUL], cdt, tag="acc")
                                dst = A[:P, :q * MUL]
                            if ts[t][2] > 0:
                                eng.tensor_add(dst, prev, tt)
                            else:
                                eng.tensor_sub(dst, prev, tt)
                            prev = dst
                        continue
                    dst1 = po[:P, PO_IDX[k] * W:PO_IDX[k] * W + q * MUL]
                    for t, (ig, b, s) in enumerate(ts):
                        o0 = slab_off[(k, t)]
                        idk = (ident if s > 0 else nident)[:P, :P]
                        nc.tensor.matmul(
                            dst1, idk, TMPS[:P, o0:o0 + q * MUL],
                            start=(t == 0), stop=(t == len(ts) - 1),
                            skip_group_check=True,
                        )
                # PSUM -> SBUF (cast to fp16) for the PE-accumulated k's
                nc.scalar.copy(out=O[:P, W:4 * W], in_=po[:P, 0:3 * W])
                nc.scalar.copy(out=O[:P, 7 * W:8 * W], in_=po[:P, 3 * W:4 * W])
                nc.gpsimd.dma_start(
                    out=out[z0:z0 + rows, :].rearrange(
                        "(q p) (k u) -> p k q u", p=P, u=MUL),
                    in_=O[:P, :].rearrange("p (k q u) -> p k q u", k=9, q=zq)[
                        :, :, :q, :],
                )
    nc.compile()
    return nc


def _get_nc(ns, zq=3, cdt_name="float16"):
    key = (ns, zq, cdt_name)
    if key not in _CACHE:
        _CACHE[key] = _build_nc(ns, zq, cdt_name)
    return _CACHE[key]


def kernel(x1, x2, w):
    from concourse.bass_utils import run_bass_kernel_spmd

    N = x1.shape[0]
    assert N % N_CORES == 0
    ns = N // N_CORES
    R = _build_R(np.asarray(w, dtype=np.float32))
    x1f = np.ascontiguousarray(np.asarray(x1, dtype=np.float32).reshape(N, FREE))
    x2t = np.ascontiguousarray(np.asarray(x2, dtype=np.float32).reshape(N, 3).T)

    nc = _get_nc(ns)
    eye = np.eye(128, dtype=np.float32)
    in_maps = [
        {
            "x1": x1f[c * ns:(c + 1) * ns],
            "x2t": np.ascontiguousarray(x2t[:, c * ns:(c + 1) * ns]),
            "rmat": R,
            "eye": eye,
        }
        for c in range(N_CORES)
    ]
    res = run_bass_kernel_spmd(nc, in_maps, list(range(N_CORES)))
    out = np.concatenate([res.results[c]["out"] for c in range(N_CORES)], axis=0)
    return out.reshape(N, 9, MUL)
